# revision 1
# baseline (speedup 1.0000x reference)
"""Trainium2 Bass kernel for CapsNet conv + dynamic-routing block.

Math note: in the reference, `pred` has a singleton MI axis, so the
softmax-weighted sum over MI is `pred` itself for any routing logits
(softmax rows sum to 1), and the `b` updates never change `c`.  The whole
module therefore reduces exactly to

    out = squash(conv2d_3x3(x2, conv_w) + conv_b)   # squash over DO

with x2 = x reshaped [B, MI*DI, H, W] and output [B, MO, H, W, DO].

Strategy: data-parallel over batch (1 image per NeuronCore, 8 cores).
The shard/prep step lays x out channel-major ([ci, pix], contiguous for
full-rate DMA) and the gather step inverts the device's output layout.
Per core:
  1. DMA x[b] [128ci, 4096pix] into a zero-padded [128ci, 66, 66] image.
  2. 3x3 conv = 9 accumulating fp32r matmuls per 512-pixel chunk
     (lhsT = W[s][ci,co] stationary, rhs = shifted window of x_pad).
  3. bias added during PSUM->SBUF copy; TensorE-transpose to [pix, co];
     squash over DO with DVE/ACT; contiguous store per chunk.
"""

from contextlib import ExitStack

import numpy as np

import concourse.bass as bass
import concourse.mybir as mybir
import concourse.tile as tile
from concourse import bacc
from concourse.bass_utils import run_bass_kernel_spmd
from concourse.masks import make_identity

B, MI, H, W, DI = 8, 8, 64, 64, 16
MO, DO = 8, 16
CI = MI * DI  # 128
CO = MO * DO  # 128
P = 128
HP, WP = H + 2, W + 2  # 66 (zero pad = 1)
NCHUNK = 8  # 512-pixel chunks per 64x64 image
EPS = 1e-7

F32 = mybir.dt.float32
F32R = mybir.dt.float32r


def _body(tc, x_in, w_in, b_in, out_d, reps=1):
    nc = tc.nc
    with ExitStack() as ctx:
        consts = ctx.enter_context(tc.tile_pool(name="consts", bufs=1))
        cpsum = ctx.enter_context(tc.tile_pool(name="cpsum", bufs=5, space="PSUM"))
        opsum = ctx.enter_context(tc.tile_pool(name="opsum", bufs=3, space="PSUM"))
        work = ctx.enter_context(tc.tile_pool(name="work", bufs=4))
        outp = ctx.enter_context(tc.tile_pool(name="outp", bufs=3))

        identity_f32 = consts.tile([P, P], F32)
        make_identity(nc, identity_f32[:])
        identity_r = consts.tile([P, P], F32R)
        nc.scalar.copy(identity_r[:], identity_f32[:])

        zeros_sb = consts.tile([P, HP], F32)
        nc.vector.memset(zeros_sb[:], 0.0)

        # padded input image [ci, hp, wp]; zero the 1-wide border.
        # fp32r tiles must be written by instructions whose output dtype is
        # float32r (walrus checkMatmultFP32r): the interior comes from a
        # f32r-to-f32r DMA, the border from ACT cast-copies of zeros.
        xpad = consts.tile([P, HP, WP], F32R)
        nc.scalar.copy(xpad[:, 0, :], zeros_sb[:])
        nc.scalar.copy(xpad[:, HP - 1, :], zeros_sb[:])
        nc.scalar.copy(xpad[:, :, 0], zeros_sb[:])
        nc.scalar.copy(xpad[:, :, WP - 1], zeros_sb[:])

        # weights: [ci, s, co] in SBUF. Issued on the ACT HWDGE ring so they
        # load in parallel with the x quarters on the SP ring.
        w_sb = consts.tile([P, 9, CO], F32R)
        nc.scalar.dma_start(w_sb[:], w_in.rearrange("s ci co -> ci s co"))

        bias_sb = consts.tile([P, 1], F32)
        nc.scalar.dma_start(bias_sb[:], b_in)

        eps_sb = consts.tile([P, 1], F32)
        nc.vector.memset(eps_sb[:], EPS)

        def load_quarter(g):
            """DMA 16 h-rows of x (contiguous source) into xpad rows 16g+1..16g+17."""
            nc.sync.dma_start(
                xpad[:, 1 + 16 * g : 17 + 16 * g, 1:65],
                x_in[:, 1024 * g : 1024 * g + 1024].rearrange(
                    "ci (r w) -> ci r w", w=W
                ),
            )

        out_sb = consts.tile([P, NCHUNK, 4, CO], F32)

        import os

        variant = os.environ.get("KVAR", "full")

        def conv_pair(c0):
            # --- conv for chunks c0, c0+1: 9 accumulating matmuls each,
            # interleaved s-outer so each weight is reused back-to-back ---
            ps0 = cpsum.tile([P, 4 * P], F32, tag="ps")
            ps1 = cpsum.tile([P, 4 * P], F32, tag="ps")
            nmm = 9 if variant != "dmaonly" else 1
            for s in range(nmm):
                kh, kw = s // 3, s % 3
                for ps, c in ((ps0, c0), (ps1, c0 + 1)):
                    rhs = xpad[:, 8 * c + kh : 8 * c + kh + 8, kw : kw + 64]
                    nc.tensor.matmul(
                        ps[:],
                        w_sb[:, s, :],
                        rhs,
                        start=(s == 0),
                        stop=(s == nmm - 1),
                    )
            return ps0, ps1

        def post(c, ps, red):
            # --- PSUM -> SBUF with bias add (on ACT: per-partition bias AP);
            # written as f32r so the f32r transpose (1.5 cyc/row) is legal ---
            s_sb = work.tile([P, 4 * P], F32R, tag="s_sb")
            nc.scalar.add(s_sb[:], ps[:], bias_sb[:])

            if variant in ("convonly", "dmaonly"):
                nc.vector.tensor_copy(
                    out_sb[:, c],
                    s_sb[:].bitcast(F32).rearrange("p (t co) -> p t co", co=CO),
                )
                return

            # --- transpose to [pix, co] ---
            so = opsum.tile([P, 4, P], F32R, tag="so")
            for t in range(4):
                nc.tensor.transpose(
                    so[:, t, :], s_sb[:, t * P : (t + 1) * P], identity_r[:]
                )
            so3 = so[:].bitcast(F32).rearrange("p t (g do) -> p (t g) do", do=DO)

            # --- squash: sum of squares into this chunk's slice of `red` ---
            sq = work.tile([P, 4, P], F32, tag="sq")
            nc.scalar.square(sq[:], so[:].bitcast(F32))
            nc.vector.tensor_reduce(
                red[:, c % 2],
                sq[:].rearrange("p t (g do) -> p (t g) do", do=DO),
                axis=mybir.AxisListType.X,
                op=mybir.AluOpType.add,
            )
            return so3

        def squash_pair(c0, red, so3_0, so3_1):
            # factor = red / ((1+red) * sqrt(red+eps)), batched for 2 chunks
            r = work.tile([P, 2, 4 * MO], F32, tag="r")
            nc.scalar.activation(
                r[:], red[:], mybir.ActivationFunctionType.Sqrt, bias=eps_sb[:]
            )
            d = work.tile([P, 2, 4 * MO], F32, tag="d")
            nc.vector.scalar_tensor_tensor(
                d[:], red[:], 1.0, r[:], mybir.AluOpType.add, mybir.AluOpType.mult
            )
            rcp = work.tile([P, 2, 4 * MO], F32, tag="rcp")
            nc.vector.reciprocal(rcp[:], d[:])
            fac = work.tile([P, 2, 4 * MO], F32, tag="fac")
            nc.vector.tensor_mul(fac[:], red[:], rcp[:])

            for i, so3 in ((0, so3_0), (1, so3_1)):
                nc.vector.tensor_mul(
                    out_sb[:, c0 + i].rearrange("p t (g do) -> p (t g) do", do=DO),
                    so3,
                    fac[:, i, :, None].to_broadcast((P, 4 * MO, DO)),
                )
                # store each chunk as soon as its values are final
                if variant != "nodma":
                    nc.sync.dma_start(out_d[:, c0 + i], out_sb[:, c0 + i])

        def one_image():
            npair = NCHUNK // 2
            if variant != "nodma":
                load_quarter(0)
                load_quarter(1)
            # pairwise: dense 18-matmul conv bursts keep the PE HAM-warm;
            # the transposes/squash for the pair follow as one group.
            for p_ in range(npair):
                if variant != "nodma" and p_ + 2 <= 3:
                    load_quarter(p_ + 2)
                ps0, ps1 = conv_pair(2 * p_)
                red = work.tile([P, 2, 4 * MO], F32, tag="red")
                so3_0 = post(2 * p_, ps0, red)
                so3_1 = post(2 * p_ + 1, ps1, red)
                if variant not in ("convonly", "dmaonly"):
                    squash_pair(2 * p_, red, so3_0, so3_1)
                elif variant != "nodma":
                    nc.sync.dma_start(
                        out_d[:, 2 * p_ : 2 * p_ + 2], out_sb[:, 2 * p_ : 2 * p_ + 2]
                    )
            if variant == "nodma":
                nc.sync.dma_start(out_d[:, 0], out_sb[:, 0])

        if reps == 1:
            one_image()
        else:
            with tc.For_i(0, reps, 1):
                one_image()


_NC_CACHE = {}


def _get_nc(reps=1):
    key = ("nc", reps)
    if key not in _NC_CACHE:
        nc = bacc.Bacc("TRN2", target_bir_lowering=False, debug=False, num_devices=8)
        x_in = nc.dram_tensor("x", [CI, H * W], F32R, kind="ExternalInput").ap()
        w_in = nc.dram_tensor("w", [9, CI, CO], F32R, kind="ExternalInput").ap()
        b_in = nc.dram_tensor("bias", [CO, 1], F32, kind="ExternalInput").ap()
        out_d = nc.dram_tensor("out", [P, NCHUNK, 4, CO], F32, kind="ExternalOutput").ap()
        with tile.TileContext(nc) as tc:
            _body(tc, x_in, w_in, b_in, out_d, reps=reps)
        nc.compile()
        _NC_CACHE[key] = nc
    return _NC_CACHE[key]


def run(x, conv_w, conv_b, trace=False, reps=1):
    nc = _get_nc(reps=reps)
    # shard/prep: channel-major x per image, [ci, pix] contiguous
    xt = np.ascontiguousarray(
        np.asarray(x, dtype=np.float32).transpose(0, 1, 4, 2, 3).reshape(B, CI, H * W)
    )
    w9 = np.ascontiguousarray(
        np.asarray(conv_w, dtype=np.float32).reshape(CO, CI, 9).transpose(2, 1, 0)
    )
    bias = np.ascontiguousarray(np.asarray(conv_b, dtype=np.float32).reshape(CO, 1))
    in_maps = [{"x": xt[b], "w": w9, "bias": bias} for b in range(B)]
    res = run_bass_kernel_spmd(nc, in_maps, list(range(B)), trace=trace)
    # gather/unshard: out_dev[p, c, t, mo, do] -> out[b, mo, h, w, do]
    # with h = 8c + 2t + p//64, w = p%64
    dev = np.stack([res.results[i]["out"] for i in range(B)], axis=0)
    dev = dev.reshape(B, 2, W, NCHUNK, 4, MO, DO)  # [b, hl, w, c, t, mo, do]
    out = np.ascontiguousarray(
        dev.transpose(0, 5, 3, 4, 1, 2, 6).reshape(B, MO, H, W, DO)
    )
    return out, res


def kernel(x, conv_w, conv_b, b_logits=None, **_ignored):
    # b_logits provably has no effect on the reference output (see module
    # docstring), so it is accepted and ignored.
    out, _ = run(x, conv_w, conv_b, trace=False)
    return out

